# revision 23
# baseline (speedup 1.0000x reference)
"""Trainium2 Bass kernel for causal multi-head attention (B=4, S=2048, E=1024, H=16).

Sharding: 8 cores = (batch b in 0..3) x (head-group g in 0..1); each core
computes one batch and 8 heads end-to-end:
  - column-parallel QKV projection (only its heads' columns)
  - causal attention for its 8 heads
  - row-parallel output projection -> partial [S, E]
The two partials per batch are summed on the host (plus b_proj). No on-device
collectives are needed.

Device dataflow (per core), all matmuls in bf16 with fp32 PSUM accumulation:
  - Q^T, K^T computed directly in [feature, token] layout (out = W^T @ x^T),
    so attention needs no transposes. K^T is pre-scaled by 1/sqrt(d)=0.125.
  - Heads are processed in PAIRS: head A lives on SBUF partitions 0..63 of
    kTs/oTs, head B on partitions 64..127. Q^T is stored per-head in
    zero-padded [128,S] tiles so every score matmul contracts the full 128
    partitions (the pad rows multiply to zero): same N-bound cost as K=64,
    but the whole program runs in one PE tiling mode (no mode-switch
    drains; PE row-tile concurrency is unreachable anyway because per-
    matmul semaphore dispatch costs ~a full N=512 stream time).
  - scores S^T[k,q]: two k-tiles fused per [128,1024] PSUM tile per head;
    one ACT exp evacuates both; causal mask via multiplicative 0/1 bf16
    masks on diagonal tiles (fully-masked tiles skipped).
  - AV: lhsT = [V | ones x 64] (M=128, same N-bound matmul cost as M=65):
    PSUM rows 0..63 = O^T (unnormalized), rows 64..127 = 64 replicated
    copies of the softmax denominator -> normalization needs no partition
    broadcast at all: a 5-op DVE Newton-reciprocal chain (bitwise seed +
    one Chebyshev-scaled NR step, ~0.17% max err) multiplies O^T in place,
    keeping the normalize entirely off the ACT queue that paces the exp
    chain. Normalize chains are deferred into the next (hp, q-block) group
    so the PE never stalls on them and the DVE PSUM read happens well
    after the AV matmul drain (direct DVE reads of freshly PE-written PSUM
    were observed to return garbage on HW).
  - All PSUM evacuation of projection groups goes through ACT (race-safe).
  - c_proj contracts the stacked O^T [512, S] against W_proj rows; its
    groups are pumped into the pair-3 attention stretch.
"""

import os
import sys

import numpy as np

for _p in ("/opt/trn_rl_repo", "/root/.axon_site/_ro/trn_rl_repo"):
    if os.path.isdir(_p) and _p not in sys.path:
        sys.path.append(_p)

import ml_dtypes  # noqa: E402

import concourse.bass as bass  # noqa: E402
import concourse.tile as tile  # noqa: E402
from concourse import bacc, bass_utils, mybir  # noqa: E402

BF16 = ml_dtypes.bfloat16

B, S, E, H = 4, 2048, 1024, 16
D = E // H            # 64
NCORE = 8
HPC = H // 2          # heads per core = 8
NPAIR = HPC // 2      # head pairs per core = 4
KT = E // 128         # contraction tiles over E = 8
TB512 = S // 512      # 4
TB128 = S // 128      # 16
FPC = HPC * D         # features per core in attention output = 512

_cache: dict = {}


def _ts(i, n):
    return slice(i * n, (i + 1) * n)


def _build_program(with_bias):
    bf = mybir.dt.bfloat16
    f32 = mybir.dt.float32
    nc = bacc.Bacc("TRN2", target_bir_lowering=False, debug=False)

    xt = nc.dram_tensor("xt", [128, KT, S], bf, kind="ExternalInput")
    wqk = nc.dram_tensor("wqk", [128, KT, 2 * FPC], bf, kind="ExternalInput")
    wv = nc.dram_tensor("wv", [128, KT, FPC], bf, kind="ExternalInput")
    wp = nc.dram_tensor("wp", [128, FPC // 128, E], bf, kind="ExternalInput")
    if with_bias:
        bqk = nc.dram_tensor("bqk", [1, 2 * FPC], bf, kind="ExternalInput")
        bv = nc.dram_tensor("bv", [1, FPC], bf, kind="ExternalInput")
    msk = nc.dram_tensor("msk", [128, 4, 512], bf, kind="ExternalInput")
    out = nc.dram_tensor("out", [S, E], f32, kind="ExternalOutput")

    Exp = mybir.ActivationFunctionType.Exp

    with tile.TileContext(nc) as tc:
        with (
            tc.tile_pool(name="big", bufs=1) as big,
            tc.tile_pool(name="pp", bufs=4) as pp,
            tc.tile_pool(name="sm", bufs=2) as sm,
            tc.tile_pool(name="ob", bufs=2) as ob,
        ):
            xts = [
                big.tile([128, S], bf, tag=f"xt{k}", name=f"xt{k}")
                for k in range(KT)
            ]
            wqks = [
                big.tile([128, 2 * FPC], bf, tag=f"wqk{k}", name=f"wqk{k}")
                for k in range(KT)
            ]
            wv_sb = big.tile([128, KT, FPC], bf, tag="wv")
            wp_sb = big.tile([128, FPC // 128, E], bf, tag="wp")
            if with_bias:
                bqk_sb = big.tile([1, 2 * FPC], bf, tag="bqk")
                bv_sb = big.tile([1, FPC], bf, tag="bv")
            msk_sb = big.tile([128, 4, 512], bf, tag="msk")
            if with_bias:
                ones_sb = big.tile([1, S], bf, tag="ones")
            # Q^T per HEAD, zero-padded to 128 partitions (head A data on
            # rows 0..63 with rows 64..127 = 0, head B data on rows 64..127
            # with rows 0..63 = 0). Score matmuls then contract over the full
            # 128 partitions of kTs (both heads) -- the zero rows kill the
            # other head's contribution, the matmul cost is N-bound so the
            # padding is free, and every matmul in the program runs in the
            # same 128-row PE mode (no mode-switch drains between scores and
            # AV/projection work).
            qzs = [big.tile([128, S], bf, tag=f"qz{h}", name=f"qz{h}") for h in range(HPC)]
            # K^T and O^T per pair: head A on partitions 0..63, B on 64..127
            kTs = [big.tile([128, S], bf, tag=f"kT{p}", name=f"kT{p}") for p in range(NPAIR)]
            oTs = [big.tile([128, S], bf, tag=f"oT{p}", name=f"oT{p}") for p in range(NPAIR)]
            # [V | ones*64] stationary tiles: cols 0..63 = V features, cols
            # 64..127 = 1.0 (denominator rows, pre-broadcast in PSUM)
            vone_k = [
                big.tile([128, 2, HPC, 2 * D], bf, tag=f"vone{tp}", name=f"vone{tp}")
                for tp in range(TB128 // 2)
            ]

            if with_bias:
                nc.sync.dma_start(out=bqk_sb, in_=bqk.ap())
                nc.sync.dma_start(out=bv_sb, in_=bv.ap())
            # first token-halves first so phase-1a's tbp=0 groups start early
            for k in range(KT):
                nc.sync.dma_start(out=wqks[k], in_=wqk.ap()[:, k, :])
                nc.sync.dma_start(out=xts[k][:, 0 : S // 2], in_=xt.ap()[:, k, 0 : S // 2])
            nc.sync.dma_start(out=wv_sb, in_=wv.ap())
            nc.sync.dma_start(out=msk_sb, in_=msk.ap())
            for k in range(KT):
                nc.sync.dma_start(
                    out=xts[k][:, S // 2 : S], in_=xt.ap()[:, k, S // 2 : S]
                )
            nc.sync.dma_start(out=wp_sb, in_=wp.ap())

            if with_bias:
                nc.vector.memset(ones_sb, 1.0)
            for tp in range(TB128 // 2):
                nc.vector.memset(vone_k[tp][:, :, :, D : 2 * D], 1.0)
            for h in range(HPC):
                pad = slice(D, 2 * D) if h % 2 == 0 else slice(0, D)
                nc.vector.memset(qzs[h][pad, :], 0.0)

            # Attention-phase PSUM pools are also used to double-buffer the
            # upfront QKV/V projection groups (same [128,1024] tile shape).
            # Bank budget: sp 2x[128,1024]=4 + av 2x[128,512]=2 + qk 2 = 8.
            with (
                tc.tile_pool(name="sp", bufs=2, space="PSUM") as ps_sp,
                tc.tile_pool(name="av", bufs=2, space="PSUM") as ps_av,
                tc.tile_pool(name="qk", bufs=1, space="PSUM") as ps_qk,
            ):

                def emit_qk_group(fb, tbp, pool, tag):
                    """One Q^T/K^T projection group: 16-18 matmuls + 1 cast.
                    Yields after each instruction so it can be interleaved."""
                    ps = pool.tile([128, 1024], f32, tag=tag, name=f"qkv_{fb}_{tbp}")
                    for half in range(2):
                        tb = 2 * tbp + half
                        hs = _ts(half, 512)
                        for kt in range(KT):
                            nc.tensor.matmul(
                                ps[:, hs],
                                lhsT=wqks[kt][:, _ts(fb, 128)],
                                rhs=xts[kt][:, _ts(tb, 512)],
                                start=(kt == 0),
                                stop=(kt == KT - 1 and not with_bias),
                            )
                            yield
                        if with_bias:
                            nc.tensor.matmul(
                                ps[:, hs],
                                lhsT=bqk_sb[0:1, _ts(fb, 128)],
                                rhs=ones_sb[0:1, _ts(tb, 512)],
                                start=False,
                                stop=True,
                            )
                            yield
                    if fb < NPAIR:
                        nc.scalar.copy(
                            qzs[2 * fb][0:D, _ts(tbp, 1024)], ps[0:D, :]
                        )
                        yield
                        nc.scalar.copy(
                            qzs[2 * fb + 1][D : 2 * D, _ts(tbp, 1024)],
                            ps[D : 2 * D, :],
                        )
                    else:
                        nc.scalar.mul(kTs[fb - NPAIR][:, _ts(tbp, 1024)], ps, 0.125)
                    yield

                # ---- Phase 1b: V = x @ Wv (+bias), [token, feature] ----
                def emit_v_group(tbp, pool, tag):
                    ps = pool.tile([128, 1024], f32, tag=tag, name=f"v_{tbp}")
                    for half in range(2):
                        tb = 2 * tbp + half
                        hs = _ts(half, 512)
                        for kt in range(KT):
                            nc.tensor.matmul(
                                ps[:, hs],
                                lhsT=xts[kt][:, _ts(tb, 128)],
                                rhs=wv_sb[:, kt, :],
                                start=(kt == 0),
                                stop=(kt == KT - 1 and not with_bias),
                            )
                            yield
                        if with_bias:
                            nc.tensor.matmul(
                                ps[:, hs],
                                lhsT=ones_sb[0:1, 0:128],
                                rhs=bv_sb[0:1, :],
                                start=False,
                                stop=True,
                            )
                            yield
                    nc.scalar.copy(
                        vone_k[tbp][:, :, :, 0:D],
                        ps[:, :].rearrange("p (t h d) -> p t h d", t=2, d=D),
                    )
                    yield

                # ---- Phase 1 schedule, ordered by DMA arrival: the tbp=0
                # QK groups and V groups 0..3 only need the first token-half
                # of x, so they run while the second half streams in; the
                # tbp=1 QK groups follow once x-h2 has landed ----
                pools = [ps_qk, ps_sp]
                gi = 0
                for kind, a, b in (
                    ("qk", 0, 0), ("qk", NPAIR, 0),
                    ("v", 0, None), ("v", 1, None), ("v", 2, None), ("v", 3, None),
                    ("qk", 0, 1), ("qk", NPAIR, 1),
                    ("v", 4, None), ("v", 5, None), ("v", 6, None), ("v", 7, None),
                ):
                    pool = pools[gi % 2]
                    tag = "qkvi" if pool is ps_qk else "sp"
                    gen = (
                        emit_qk_group(a, b, pool, tag)
                        if kind == "qk"
                        else emit_v_group(a, pool, tag)
                    )
                    for _ in gen:
                        pass
                    gi += 1

                # ---- feed: QKV pairs 1..3, interleaved into attention to
                # keep the PE dense while ACT paces the exp chain ----
                def main_feed():
                    for pr in range(1, NPAIR):
                        for fb in (pr, pr + NPAIR):
                            for tbp in range(TB512 // 2):
                                yield from (
                                    pr for _ in emit_qk_group(fb, tbp, ps_qk, "qkvi")
                                )

                feed = main_feed()
                # cur_pr semantics: pairs < cur_pr are fully emitted
                feed_state = {"cur_pr": 1, "alive": True}

                def pump(n):
                    for _ in range(n):
                        got = next(feed, None)
                        if got is None:
                            feed_state["alive"] = False
                            feed_state["cur_pr"] = NPAIR + 1
                            return
                        feed_state["cur_pr"] = got

                def drain_feed_through(pr):
                    """Emit QKV work until every pair <= pr is complete.
                    Required before attention reads qTs[pr]/kTs[pr]: Tile only
                    sees RAW deps for writes emitted before the read."""
                    while feed_state["alive"] and feed_state["cur_pr"] <= pr:
                        pump(1)

                # ---- c_proj feed: pumped into pair-3 attention as the QKV
                # feed runs dry there. A tb group unlocks once every head has
                # normalized its q-block (pair 3 / head B is the last writer).
                def cproj_group(tb):
                    ps = ps_qk.tile([128, 1024], f32, tag="qkvi", name=f"pj{tb}")
                    for eb in range(2):
                        for fg in range(FPC // 128):
                            nc.tensor.matmul(
                                ps[:, _ts(eb, 512)],
                                lhsT=oTs[fg][:, _ts(tb, 128)],
                                rhs=wp_sb[:, fg, _ts(eb, 512)],
                                start=(fg == 0),
                                stop=(fg == FPC // 128 - 1),
                            )
                            yield
                    o_sb = ob.tile([128, 1024], f32, tag="osb", name=f"ob{tb}")
                    nc.scalar.copy(o_sb, ps)
                    yield
                    nc.sync.dma_start(out=out.ap()[_ts(tb, 128), :], in_=o_sb)
                    yield

                # tb groups are emitted in pair-3's (descending) qb order so
                # only the tiny qb=0 group's cproj work trails at the tail
                def cproj_feed():
                    for qb_grp in reversed(range(TB512)):
                        while qb_grp not in cp_state["unlocked"]:
                            yield False  # not allowed yet; no emission
                        for tb in range(4 * qb_grp, 4 * qb_grp + 4):
                            yield from (True for _ in cproj_group(tb))

                cp_state = {"unlocked": set()}
                cfeed = cproj_feed()

                def pump_cproj(n):
                    for _ in range(n):
                        got = next(cfeed, None)
                        if got is None or got is False:
                            return

                # ---- Phase 2: causal attention per head pair ----
                deferred = []  # pending normalize chains (one per head/group)

                def normalize(av, hp, qoff, qb):
                    # av rows 64..127 hold 64 replicated copies of the
                    # softmax denominator, so no partition broadcast is
                    # needed. 1/denom via a 5-op DVE Newton chain (bitwise
                    # exponent-flip seed + Chebyshev-scaled NR step, ~0.17%
                    # max err) -- keeps the normalize entirely OFF the ACT
                    # queue, which paces the exp chain the PE is coupled to.
                    # All reads of av are deferred a group behind the AV
                    # matmuls (direct DVE reads of freshly PE-written PSUM
                    # return garbage on HW). x = av[D:], num = av[:D].
                    i32 = mybir.dt.int32
                    x = av[D : 2 * D, :]
                    nx = sm.tile([D, 512], f32, tag="nx")
                    y0 = sm.tile([D, 512], f32, tag="y0")
                    tt = sm.tile([D, 512], f32, tag="tt")
                    zz = sm.tile([D, 512], f32, tag="zz")
                    nc.vector.tensor_scalar(
                        nx.bitcast(i32), x.bitcast(i32), -1, None,
                        mybir.AluOpType.bitwise_xor,
                    )
                    nc.vector.tensor_scalar(
                        y0, nx, -0.23549792, None, mybir.AluOpType.mult
                    )
                    nc.vector.tensor_tensor(tt, x, y0, mybir.AluOpType.mult)
                    nc.vector.scalar_tensor_tensor(
                        zz, tt, 2.0017324, y0,
                        mybir.AluOpType.subtract, mybir.AluOpType.mult,
                    )
                    nc.vector.scalar_tensor_tensor(
                        oTs[hp][qoff : qoff + D, _ts(qb, 512)],
                        av[0:D, :], -1.0, zz,
                        mybir.AluOpType.mult, mybir.AluOpType.mult,
                    )

                for hp in range(NPAIR):
                    drain_feed_through(hp)
                    # pair 3 runs its q-blocks big-to-small so the cproj
                    # groups that depend on each qb unlock as early as
                    # possible and only qb=0's trail at the end
                    qb_order = range(TB512)
                    if hp == NPAIR - 1:
                        qb_order = reversed(range(TB512))
                    for qb in qb_order:
                        npairs = 2 * qb + 2  # fused ki-pairs (4qb+4 k-tiles)
                        avs = [
                            ps_av.tile([128, 512], f32, tag="av", name=f"av{hd}")
                            for hd in range(2)
                        ]
                        # pop deferred normalizes BEFORE any AV matmul of this
                        # group: the av pool rotates every group, so the WAR
                        # dep (new AV write vs old normalize read) must see
                        # the read emitted first.
                        while deferred:
                            fn, dhp, dhd, dqb = deferred.pop(0)
                            fn()
                            if dhp == NPAIR - 1 and dhd == 1:
                                cp_state["unlocked"].add(dqb)
                        pend = None

                        def do_av(p, p_sbs, avs=avs, hp=hp, npairs=npairs):
                            for hd in range(2):
                                h = 2 * hp + hd
                                for half in range(2):
                                    ki = 2 * p + half
                                    nc.tensor.matmul(
                                        avs[hd],
                                        lhsT=vone_k[ki // 2][:, ki % 2, h, :],
                                        rhs=p_sbs[hd][:, _ts(half, 512)],
                                        start=(ki == 0),
                                        stop=(ki == 2 * npairs - 1),
                                    )

                        for p in range(npairs):
                            sps = [
                                ps_sp.tile([128, 1024], f32, tag="sp", name=f"sp{hd}")
                                for hd in range(2)
                            ]
                            # K=128 full-partition scores: kTs carries both
                            # heads; the zero-padded qzs tile selects one.
                            # Same N-bound cost as K=64 but no PE mode/row-
                            # group switches anywhere in the program.
                            for hd in range(2):
                                for half in range(2):
                                    ki = 2 * p + half
                                    nc.tensor.matmul(
                                        sps[hd][:, _ts(half, 512)],
                                        lhsT=kTs[hp][:, _ts(ki, 128)],
                                        rhs=qzs[2 * hp + hd][:, _ts(qb, 512)],
                                        start=True,
                                        stop=True,
                                    )
                            if pend is not None:
                                do_av(*pend)
                                pump(4)
                            else:
                                pump(6)
                            p_sbs = [
                                pp.tile([128, 1024], bf, tag="p", name=f"p{hd}")
                                for hd in range(2)
                            ]
                            for hd in range(2):
                                nc.scalar.activation(p_sbs[hd], sps[hd], Exp)
                            if p >= 2 * qb:  # both halves are diagonal tiles
                                j = 2 * (p - 2 * qb)
                                for hd in range(2):
                                    nc.vector.tensor_mul(
                                        p_sbs[hd],
                                        p_sbs[hd],
                                        msk_sb[:, j : j + 2, :].rearrange(
                                            "k j q -> k (j q)"
                                        ),
                                    )
                            pend = (p, p_sbs)
                            if hp == NPAIR - 1:
                                # qb3 runs first and its 8 steps pump against
                                # a still-locked feed; the later, shorter qb
                                # groups need a high rate to drain all 16 tb
                                # groups before the loop ends
                                pump_cproj(16)
                        do_av(*pend)
                        for hd in range(2):
                            deferred.append(
                                (
                                    lambda av=avs[hd], hp=hp, qoff=hd * D, qb=qb: normalize(
                                        av, hp, qoff, qb
                                    ),
                                    hp,
                                    hd,
                                    qb,
                                )
                            )
                while feed_state["alive"]:  # drain any remaining feed
                    pump(1)
                while deferred:
                    fn, dhp, dhd, dqb = deferred.pop(0)
                    fn()
                cp_state["unlocked"] = set(range(TB512))
                while next(cfeed, None) is not None:
                    pass

    nc.compile()
    return nc


def _part_major(a, p=128):
    """[n*128, m] -> [128, n, m] with partition index innermost in rows."""
    n = a.shape[0] // p
    return np.ascontiguousarray(a.reshape(n, p, a.shape[1]).transpose(1, 0, 2))


def make_in_maps(x, W_attn, b_attn, W_proj, with_bias=False):
    """Build the 8 per-core input maps (core = 2*b + g)."""
    x = np.asarray(x, dtype=np.float32)
    W_attn = np.asarray(W_attn, dtype=np.float32)
    b_attn = np.asarray(b_attn, dtype=np.float32)
    W_proj = np.asarray(W_proj, dtype=np.float32)

    # causal 0/1 masks for the 4 diagonal alignments (k-tile 128 vs q-block 512)
    kk = np.arange(128)[:, None]
    qq = np.arange(512)[None, :]
    msk = np.stack(
        [(qq >= j * 128 + kk) for j in range(4)], axis=1
    ).astype(BF16)  # [128, 4, 512]

    in_maps = []
    for b in range(B):
        xt = _part_major(np.ascontiguousarray(x[b].T)).astype(BF16)  # [128,8,S]
        for g in range(2):
            qs = W_attn[:, g * FPC : (g + 1) * FPC]
            ks = W_attn[:, E + g * FPC : E + (g + 1) * FPC]
            vs = W_attn[:, 2 * E + g * FPC : 2 * E + (g + 1) * FPC]
            wqk = _part_major(np.concatenate([qs, ks], axis=1)).astype(BF16)
            wv = _part_major(vs).astype(BF16)
            wp = _part_major(W_proj[g * FPC : (g + 1) * FPC, :]).astype(BF16)
            bq = b_attn[g * FPC : (g + 1) * FPC]
            bk = b_attn[E + g * FPC : E + (g + 1) * FPC]
            bqk = np.concatenate([bq, bk])[None, :].astype(BF16)
            bv = b_attn[2 * E + g * FPC : 2 * E + (g + 1) * FPC][None, :].astype(
                BF16
            )
            m = {
                "xt": xt,
                "wqk": np.ascontiguousarray(wqk),
                "wv": np.ascontiguousarray(wv),
                "wp": np.ascontiguousarray(wp),
                "msk": np.ascontiguousarray(msk),
            }
            if with_bias:
                m["bqk"] = np.ascontiguousarray(bqk)
                m["bv"] = np.ascontiguousarray(bv)
            in_maps.append(m)
    return in_maps


def get_program(with_bias=False):
    key = f"nc{int(with_bias)}"
    if key not in _cache:
        _cache[key] = _build_program(with_bias)
    return _cache[key]


def gather(results, b_proj):
    b_proj = np.asarray(b_proj, dtype=np.float32)
    out = np.empty((B, S, E), dtype=np.float32)
    for b in range(B):
        out[b] = results[2 * b]["out"] + results[2 * b + 1]["out"] + b_proj
    return out


def kernel(x, W_attn, b_attn, W_proj, b_proj):
    with_bias = bool(np.any(np.asarray(b_attn)))
    nc = get_program(with_bias)
    in_maps = make_in_maps(x, W_attn, b_attn, W_proj, with_bias=with_bias)
    res = bass_utils.run_bass_kernel_spmd(nc, in_maps, core_ids=list(range(NCORE)))
    return gather(res.results, b_proj)


# revision 25
# speedup vs baseline: 1.1663x; 1.1663x over previous
"""Trainium2 Bass kernel for causal multi-head attention (B=4, S=2048, E=1024, H=16).

Sharding: 8 cores = (batch b in 0..3) x (head-group g in 0..1); each core
computes one batch and 8 heads end-to-end:
  - column-parallel QKV projection (only its heads' columns)
  - causal attention for its 8 heads
  - row-parallel output projection -> partial [S, E]
The two partials per batch are summed on the host (plus b_proj). No on-device
collectives are needed.

Device dataflow (per core), all matmuls in bf16 with fp32 PSUM accumulation:
  - Q^T, K^T computed directly in [feature, token] layout (out = W^T @ x^T),
    so attention needs no transposes. K^T is pre-scaled by 1/sqrt(d)=0.125.
  - Heads are processed in PAIRS: head A lives on SBUF partitions 0..63 of
    kTs/oTs, head B on partitions 64..127. Q^T is stored per-head in
    zero-padded [128,S] tiles so every score matmul contracts the full 128
    partitions (the pad rows multiply to zero): same N-bound cost as K=64,
    but the whole program runs in one PE tiling mode (no mode-switch
    drains; PE row-tile concurrency is unreachable anyway because per-
    matmul semaphore dispatch costs ~a full N=512 stream time).
  - scores S^T[k,q]: two k-tiles fused per [128,1024] PSUM tile per head;
    one ACT exp evacuates both; causal mask via multiplicative 0/1 bf16
    masks on diagonal tiles (fully-masked tiles skipped).
  - AV: lhsT = [V | ones x 64] (M=128, same N-bound matmul cost as M=65):
    PSUM rows 0..63 = O^T (unnormalized), rows 64..127 = 64 replicated
    copies of the softmax denominator -> normalization needs no partition
    broadcast at all: a 5-op DVE Newton-reciprocal chain (bitwise seed +
    one Chebyshev-scaled NR step, ~0.17% max err) multiplies O^T in place,
    keeping the normalize entirely off the ACT queue that paces the exp
    chain. Normalize chains are deferred into the next (hp, q-block) group
    so the PE never stalls on them and the DVE PSUM read happens well
    after the AV matmul drain (direct DVE reads of freshly PE-written PSUM
    were observed to return garbage on HW).
  - All PSUM evacuation of projection groups goes through ACT (race-safe).
  - c_proj contracts the stacked O^T [512, S] against W_proj rows; its
    groups are pumped into the pair-3 attention stretch.
"""

import os
import sys

import numpy as np

for _p in ("/opt/trn_rl_repo", "/root/.axon_site/_ro/trn_rl_repo"):
    if os.path.isdir(_p) and _p not in sys.path:
        sys.path.append(_p)

import ml_dtypes  # noqa: E402

import concourse.bass as bass  # noqa: E402
import concourse.tile as tile  # noqa: E402
from concourse import bacc, bass_utils, mybir  # noqa: E402

BF16 = ml_dtypes.bfloat16

B, S, E, H = 4, 2048, 1024, 16
D = E // H            # 64
NCORE = 8
HPC = H // 2          # heads per core = 8
NPAIR = HPC // 2      # head pairs per core = 4
KT = E // 128         # contraction tiles over E = 8
TB512 = S // 512      # 4
TB128 = S // 128      # 16
FPC = HPC * D         # features per core in attention output = 512

_cache: dict = {}


def _ts(i, n):
    return slice(i * n, (i + 1) * n)


def _build_program(with_bias):
    bf = mybir.dt.bfloat16
    f32 = mybir.dt.float32
    nc = bacc.Bacc("TRN2", target_bir_lowering=False, debug=False)

    xt = nc.dram_tensor("xt", [128, KT, S], bf, kind="ExternalInput")
    wqk = nc.dram_tensor("wqk", [128, KT, 2 * FPC], bf, kind="ExternalInput")
    wv = nc.dram_tensor("wv", [128, KT, FPC], bf, kind="ExternalInput")
    wp = nc.dram_tensor("wp", [128, FPC // 128, E], bf, kind="ExternalInput")
    if with_bias:
        bqk = nc.dram_tensor("bqk", [1, 2 * FPC], bf, kind="ExternalInput")
        bv = nc.dram_tensor("bv", [1, FPC], bf, kind="ExternalInput")
    msk = nc.dram_tensor("msk", [128, 4, 512], bf, kind="ExternalInput")
    out = nc.dram_tensor("out", [S, E], f32, kind="ExternalOutput")

    Exp = mybir.ActivationFunctionType.Exp

    with tile.TileContext(nc) as tc:
        with (
            tc.tile_pool(name="big", bufs=1) as big,
            tc.tile_pool(name="pp", bufs=4) as pp,
            tc.tile_pool(name="sm", bufs=2) as sm,
            tc.tile_pool(name="ob", bufs=2) as ob,
        ):
            xts = [
                big.tile([128, S], bf, tag=f"xt{k}", name=f"xt{k}")
                for k in range(KT)
            ]
            wqks = [
                big.tile([128, 2 * FPC], bf, tag=f"wqk{k}", name=f"wqk{k}")
                for k in range(KT)
            ]
            wv_sb = big.tile([128, KT, FPC], bf, tag="wv")
            wp_sb = big.tile([128, FPC // 128, E], bf, tag="wp")
            if with_bias:
                bqk_sb = big.tile([1, 2 * FPC], bf, tag="bqk")
                bv_sb = big.tile([1, FPC], bf, tag="bv")
            msk_sb = big.tile([128, 4, 512], bf, tag="msk")
            if with_bias:
                ones_sb = big.tile([1, S], bf, tag="ones")
            # Q^T per HEAD, zero-padded to 128 partitions (head A data on
            # rows 0..63 with rows 64..127 = 0, head B data on rows 64..127
            # with rows 0..63 = 0). Score matmuls then contract over the full
            # 128 partitions of kTs (both heads) -- the zero rows kill the
            # other head's contribution, the matmul cost is N-bound so the
            # padding is free, and every matmul in the program runs in the
            # same 128-row PE mode (no mode-switch drains between scores and
            # AV/projection work).
            qzs = [big.tile([128, S], bf, tag=f"qz{h}", name=f"qz{h}") for h in range(HPC)]
            # K^T and O^T per pair: head A on partitions 0..63, B on 64..127
            kTs = [big.tile([128, S], bf, tag=f"kT{p}", name=f"kT{p}") for p in range(NPAIR)]
            oTs = [big.tile([128, S], bf, tag=f"oT{p}", name=f"oT{p}") for p in range(NPAIR)]
            # [V | ones*64] stationary tiles: cols 0..63 = V features, cols
            # 64..127 = 1.0 (denominator rows, pre-broadcast in PSUM)
            vone_k = [
                big.tile([128, 2, HPC, 2 * D], bf, tag=f"vone{tp}", name=f"vone{tp}")
                for tp in range(TB128 // 2)
            ]

            if with_bias:
                nc.sync.dma_start(out=bqk_sb, in_=bqk.ap())
                nc.sync.dma_start(out=bv_sb, in_=bv.ap())
            # first token-halves first so phase-1a's tbp=0 groups start early
            for k in range(KT):
                nc.sync.dma_start(out=wqks[k], in_=wqk.ap()[:, k, :])
                nc.sync.dma_start(out=xts[k][:, 0 : S // 2], in_=xt.ap()[:, k, 0 : S // 2])
            nc.sync.dma_start(out=wv_sb, in_=wv.ap())
            nc.sync.dma_start(out=msk_sb, in_=msk.ap())
            for k in range(KT):
                nc.sync.dma_start(
                    out=xts[k][:, S // 2 : S], in_=xt.ap()[:, k, S // 2 : S]
                )
            nc.sync.dma_start(out=wp_sb, in_=wp.ap())

            if with_bias:
                nc.vector.memset(ones_sb, 1.0)
            for tp in range(TB128 // 2):
                nc.vector.memset(vone_k[tp][:, :, :, D : 2 * D], 1.0)
            for h in range(HPC):
                pad = slice(D, 2 * D) if h % 2 == 0 else slice(0, D)
                nc.vector.memset(qzs[h][pad, :], 0.0)

            # Attention-phase PSUM pools are also used to double-buffer the
            # upfront QKV/V projection groups (same [128,1024] tile shape).
            # Bank budget: sp 2x[128,1024]=4 + av 2x[128,512]=2 + qk 2 = 8.
            with (
                tc.tile_pool(name="sp", bufs=2, space="PSUM") as ps_sp,
                tc.tile_pool(name="av", bufs=2, space="PSUM") as ps_av,
                tc.tile_pool(name="qk", bufs=1, space="PSUM") as ps_qk,
            ):

                def emit_qk_group(fb, tbp, pool, tag):
                    """One Q^T/K^T projection group: 16-18 matmuls + 1 cast.
                    Yields after each instruction so it can be interleaved."""
                    ps = pool.tile([128, 1024], f32, tag=tag, name=f"qkv_{fb}_{tbp}")
                    for half in range(2):
                        tb = 2 * tbp + half
                        hs = _ts(half, 512)
                        for kt in range(KT):
                            nc.tensor.matmul(
                                ps[:, hs],
                                lhsT=wqks[kt][:, _ts(fb, 128)],
                                rhs=xts[kt][:, _ts(tb, 512)],
                                start=(kt == 0),
                                stop=(kt == KT - 1 and not with_bias),
                            )
                            yield
                        if with_bias:
                            nc.tensor.matmul(
                                ps[:, hs],
                                lhsT=bqk_sb[0:1, _ts(fb, 128)],
                                rhs=ones_sb[0:1, _ts(tb, 512)],
                                start=False,
                                stop=True,
                            )
                            yield
                    if fb < NPAIR:
                        nc.scalar.copy(
                            qzs[2 * fb][0:D, _ts(tbp, 1024)], ps[0:D, :]
                        )
                        yield
                        nc.scalar.copy(
                            qzs[2 * fb + 1][D : 2 * D, _ts(tbp, 1024)],
                            ps[D : 2 * D, :],
                        )
                    else:
                        nc.scalar.mul(kTs[fb - NPAIR][:, _ts(tbp, 1024)], ps, 0.125)
                    yield

                # ---- Phase 1b: V = x @ Wv (+bias), [token, feature] ----
                def emit_v_group(tbp, pool, tag):
                    ps = pool.tile([128, 1024], f32, tag=tag, name=f"v_{tbp}")
                    for half in range(2):
                        tb = 2 * tbp + half
                        hs = _ts(half, 512)
                        for kt in range(KT):
                            nc.tensor.matmul(
                                ps[:, hs],
                                lhsT=xts[kt][:, _ts(tb, 128)],
                                rhs=wv_sb[:, kt, :],
                                start=(kt == 0),
                                stop=(kt == KT - 1 and not with_bias),
                            )
                            yield
                        if with_bias:
                            nc.tensor.matmul(
                                ps[:, hs],
                                lhsT=ones_sb[0:1, 0:128],
                                rhs=bv_sb[0:1, :],
                                start=False,
                                stop=True,
                            )
                            yield
                    nc.scalar.copy(
                        vone_k[tbp][:, :, :, 0:D],
                        ps[:, :].rearrange("p (t h d) -> p t h d", t=2, d=D),
                    )
                    yield

                # ---- Phase 1: pair-0 Q^T/K^T upfront (tbp-major so the
                # first groups only need the first token-half DMAs), then V
                # upfront (dense PE work) ----
                pools = [ps_qk, ps_sp]
                gi = 0
                for tbp in range(TB512 // 2):
                    for fb in (0, NPAIR):
                        pool = pools[gi % 2]
                        for _ in emit_qk_group(
                            fb, tbp, pool, "qkvi" if pool is ps_qk else "sp"
                        ):
                            pass
                        gi += 1
                for tbp in range(TB128 // 2):
                    pool = pools[tbp % 2]
                    for _ in emit_v_group(
                        tbp, pool, "qkvi" if pool is ps_qk else "sp"
                    ):
                        pass

                # ---- feed: QKV pairs 1..3, interleaved into attention to
                # keep the PE dense while ACT paces the exp chain ----
                def main_feed():
                    for pr in range(1, NPAIR):
                        for fb in (pr, pr + NPAIR):
                            for tbp in range(TB512 // 2):
                                yield from (
                                    pr for _ in emit_qk_group(fb, tbp, ps_qk, "qkvi")
                                )

                feed = main_feed()
                # cur_pr semantics: pairs < cur_pr are fully emitted
                feed_state = {"cur_pr": 1, "alive": True}

                def pump(n):
                    for _ in range(n):
                        got = next(feed, None)
                        if got is None:
                            feed_state["alive"] = False
                            feed_state["cur_pr"] = NPAIR + 1
                            return
                        feed_state["cur_pr"] = got

                def drain_feed_through(pr):
                    """Emit QKV work until every pair <= pr is complete.
                    Required before attention reads qTs[pr]/kTs[pr]: Tile only
                    sees RAW deps for writes emitted before the read."""
                    while feed_state["alive"] and feed_state["cur_pr"] <= pr:
                        pump(1)

                # ---- c_proj feed: pumped into pair-3 attention as the QKV
                # feed runs dry there. A tb group unlocks once every head has
                # normalized its q-block (pair 3 / head B is the last writer).
                def cproj_group(tb):
                    ps = ps_qk.tile([128, 1024], f32, tag="qkvi", name=f"pj{tb}")
                    for eb in range(2):
                        for fg in range(FPC // 128):
                            nc.tensor.matmul(
                                ps[:, _ts(eb, 512)],
                                lhsT=oTs[fg][:, _ts(tb, 128)],
                                rhs=wp_sb[:, fg, _ts(eb, 512)],
                                start=(fg == 0),
                                stop=(fg == FPC // 128 - 1),
                            )
                            yield
                    o_sb = ob.tile([128, 1024], f32, tag="osb", name=f"ob{tb}")
                    nc.scalar.copy(o_sb, ps)
                    yield
                    nc.sync.dma_start(out=out.ap()[_ts(tb, 128), :], in_=o_sb)
                    yield

                # tb groups are emitted in pair-3's (descending) qb order so
                # only the tiny qb=0 group's cproj work trails at the tail
                def cproj_feed():
                    for qb_grp in reversed(range(TB512)):
                        while qb_grp not in cp_state["unlocked"]:
                            yield False  # not allowed yet; no emission
                        for tb in range(4 * qb_grp, 4 * qb_grp + 4):
                            yield from (True for _ in cproj_group(tb))

                cp_state = {"unlocked": set()}
                cfeed = cproj_feed()

                def pump_cproj(n):
                    for _ in range(n):
                        got = next(cfeed, None)
                        if got is None or got is False:
                            return

                # ---- Phase 2: causal attention per head pair ----
                deferred = []  # pending normalize chains (one per head/group)

                def normalize(av, hp, qoff, qb):
                    # av rows 64..127 hold 64 replicated copies of the
                    # softmax denominator, so no partition broadcast is
                    # needed. 1/denom via a 5-op DVE Newton chain (bitwise
                    # exponent-flip seed + Chebyshev-scaled NR step, ~0.17%
                    # max err) -- keeps the normalize entirely OFF the ACT
                    # queue, which paces the exp chain the PE is coupled to.
                    # All reads of av are deferred a group behind the AV
                    # matmuls (direct DVE reads of freshly PE-written PSUM
                    # return garbage on HW). x = av[D:], num = av[:D].
                    i32 = mybir.dt.int32
                    x = av[D : 2 * D, :]
                    nx = sm.tile([D, 512], f32, tag="nx")
                    y0 = sm.tile([D, 512], f32, tag="y0")
                    tt = sm.tile([D, 512], f32, tag="tt")
                    zz = sm.tile([D, 512], f32, tag="zz")
                    nc.vector.tensor_scalar(
                        nx.bitcast(i32), x.bitcast(i32), -1, None,
                        mybir.AluOpType.bitwise_xor,
                    )
                    nc.vector.tensor_scalar(
                        y0, nx, -0.23549792, None, mybir.AluOpType.mult
                    )
                    nc.vector.tensor_tensor(tt, x, y0, mybir.AluOpType.mult)
                    nc.vector.scalar_tensor_tensor(
                        zz, tt, 2.0017324, y0,
                        mybir.AluOpType.subtract, mybir.AluOpType.mult,
                    )
                    nc.vector.scalar_tensor_tensor(
                        oTs[hp][qoff : qoff + D, _ts(qb, 512)],
                        av[0:D, :], -1.0, zz,
                        mybir.AluOpType.mult, mybir.AluOpType.mult,
                    )

                for hp in range(NPAIR):
                    drain_feed_through(hp)
                    # pair 3 runs its q-blocks big-to-small so the cproj
                    # groups that depend on each qb unlock as early as
                    # possible and only qb=0's trail at the end
                    qb_order = range(TB512)
                    if hp == NPAIR - 1:
                        qb_order = reversed(range(TB512))
                    for qb in qb_order:
                        npairs = 2 * qb + 2  # fused ki-pairs (4qb+4 k-tiles)
                        avs = [
                            ps_av.tile([128, 512], f32, tag="av", name=f"av{hd}")
                            for hd in range(2)
                        ]
                        # pop deferred normalizes BEFORE any AV matmul of this
                        # group: the av pool rotates every group, so the WAR
                        # dep (new AV write vs old normalize read) must see
                        # the read emitted first.
                        while deferred:
                            fn, dhp, dhd, dqb = deferred.pop(0)
                            fn()
                            if dhp == NPAIR - 1 and dhd == 1:
                                cp_state["unlocked"].add(dqb)
                        pend = None

                        def do_av(p, p_sbs, avs=avs, hp=hp, npairs=npairs):
                            for hd in range(2):
                                h = 2 * hp + hd
                                for half in range(2):
                                    ki = 2 * p + half
                                    nc.tensor.matmul(
                                        avs[hd],
                                        lhsT=vone_k[ki // 2][:, ki % 2, h, :],
                                        rhs=p_sbs[hd][:, _ts(half, 512)],
                                        start=(ki == 0),
                                        stop=(ki == 2 * npairs - 1),
                                    )

                        for p in range(npairs):
                            sps = [
                                ps_sp.tile([128, 1024], f32, tag="sp", name=f"sp{hd}")
                                for hd in range(2)
                            ]
                            # K=128 full-partition scores: kTs carries both
                            # heads; the zero-padded qzs tile selects one.
                            # Same N-bound cost as K=64 but no PE mode/row-
                            # group switches anywhere in the program.
                            for hd in range(2):
                                for half in range(2):
                                    ki = 2 * p + half
                                    nc.tensor.matmul(
                                        sps[hd][:, _ts(half, 512)],
                                        lhsT=kTs[hp][:, _ts(ki, 128)],
                                        rhs=qzs[2 * hp + hd][:, _ts(qb, 512)],
                                        start=True,
                                        stop=True,
                                    )
                            if pend is not None:
                                do_av(*pend)
                                pump(4)
                            else:
                                pump(6)
                            p_sbs = [
                                pp.tile([128, 1024], bf, tag="p", name=f"p{hd}")
                                for hd in range(2)
                            ]
                            for hd in range(2):
                                nc.scalar.activation(p_sbs[hd], sps[hd], Exp)
                            if p >= 2 * qb:  # both halves are diagonal tiles
                                j = 2 * (p - 2 * qb)
                                for hd in range(2):
                                    nc.vector.tensor_mul(
                                        p_sbs[hd],
                                        p_sbs[hd],
                                        msk_sb[:, j : j + 2, :].rearrange(
                                            "k j q -> k (j q)"
                                        ),
                                    )
                            pend = (p, p_sbs)
                            if hp == NPAIR - 1:
                                pump_cproj(8)
                        do_av(*pend)
                        for hd in range(2):
                            deferred.append(
                                (
                                    lambda av=avs[hd], hp=hp, qoff=hd * D, qb=qb: normalize(
                                        av, hp, qoff, qb
                                    ),
                                    hp,
                                    hd,
                                    qb,
                                )
                            )
                while feed_state["alive"]:  # drain any remaining feed
                    pump(1)
                while deferred:
                    fn, dhp, dhd, dqb = deferred.pop(0)
                    fn()
                cp_state["unlocked"] = set(range(TB512))
                while next(cfeed, None) is not None:
                    pass

    nc.compile()
    return nc


def _part_major(a, p=128):
    """[n*128, m] -> [128, n, m] with partition index innermost in rows."""
    n = a.shape[0] // p
    return np.ascontiguousarray(a.reshape(n, p, a.shape[1]).transpose(1, 0, 2))


def make_in_maps(x, W_attn, b_attn, W_proj, with_bias=False):
    """Build the 8 per-core input maps (core = 2*b + g)."""
    x = np.asarray(x, dtype=np.float32)
    W_attn = np.asarray(W_attn, dtype=np.float32)
    b_attn = np.asarray(b_attn, dtype=np.float32)
    W_proj = np.asarray(W_proj, dtype=np.float32)

    # causal 0/1 masks for the 4 diagonal alignments (k-tile 128 vs q-block 512)
    kk = np.arange(128)[:, None]
    qq = np.arange(512)[None, :]
    msk = np.stack(
        [(qq >= j * 128 + kk) for j in range(4)], axis=1
    ).astype(BF16)  # [128, 4, 512]

    in_maps = []
    for b in range(B):
        xt = _part_major(np.ascontiguousarray(x[b].T)).astype(BF16)  # [128,8,S]
        for g in range(2):
            qs = W_attn[:, g * FPC : (g + 1) * FPC]
            ks = W_attn[:, E + g * FPC : E + (g + 1) * FPC]
            vs = W_attn[:, 2 * E + g * FPC : 2 * E + (g + 1) * FPC]
            wqk = _part_major(np.concatenate([qs, ks], axis=1)).astype(BF16)
            wv = _part_major(vs).astype(BF16)
            wp = _part_major(W_proj[g * FPC : (g + 1) * FPC, :]).astype(BF16)
            bq = b_attn[g * FPC : (g + 1) * FPC]
            bk = b_attn[E + g * FPC : E + (g + 1) * FPC]
            bqk = np.concatenate([bq, bk])[None, :].astype(BF16)
            bv = b_attn[2 * E + g * FPC : 2 * E + (g + 1) * FPC][None, :].astype(
                BF16
            )
            m = {
                "xt": xt,
                "wqk": np.ascontiguousarray(wqk),
                "wv": np.ascontiguousarray(wv),
                "wp": np.ascontiguousarray(wp),
                "msk": np.ascontiguousarray(msk),
            }
            if with_bias:
                m["bqk"] = np.ascontiguousarray(bqk)
                m["bv"] = np.ascontiguousarray(bv)
            in_maps.append(m)
    return in_maps


def get_program(with_bias=False):
    key = f"nc{int(with_bias)}"
    if key not in _cache:
        _cache[key] = _build_program(with_bias)
    return _cache[key]


def gather(results, b_proj):
    b_proj = np.asarray(b_proj, dtype=np.float32)
    out = np.empty((B, S, E), dtype=np.float32)
    for b in range(B):
        out[b] = results[2 * b]["out"] + results[2 * b + 1]["out"] + b_proj
    return out


def kernel(x, W_attn, b_attn, W_proj, b_proj):
    with_bias = bool(np.any(np.asarray(b_attn)))
    nc = get_program(with_bias)
    in_maps = make_in_maps(x, W_attn, b_attn, W_proj, with_bias=with_bias)
    res = bass_utils.run_bass_kernel_spmd(nc, in_maps, core_ids=list(range(NCORE)))
    return gather(res.results, b_proj)


# revision 29
# speedup vs baseline: 1.2296x; 1.0543x over previous
"""Trainium2 Bass kernel for causal multi-head attention (B=4, S=2048, E=1024, H=16).

Sharding: 8 cores = (batch b in 0..3) x (head-group g in 0..1); each core
computes one batch and 8 heads end-to-end:
  - column-parallel QKV projection (only its heads' columns)
  - causal attention for its 8 heads
  - row-parallel output projection -> partial [S, E]
The two partials per batch are summed on the host (plus b_proj). No on-device
collectives are needed.

Device dataflow (per core), all matmuls in bf16 with fp32 PSUM accumulation:
  - Q^T, K^T computed directly in [feature, token] layout (out = W^T @ x^T),
    so attention needs no transposes. K^T is pre-scaled by 1/sqrt(d)=0.125.
  - Heads are processed in PAIRS: head A lives on SBUF partitions 0..63 of
    kTs/oTs, head B on partitions 64..127. Q^T is stored per-head in
    zero-padded [128,S] tiles so every score matmul contracts the full 128
    partitions (the pad rows multiply to zero): same N-bound cost as K=64,
    but the whole program runs in one PE tiling mode (no mode-switch
    drains; PE row-tile concurrency is unreachable anyway because per-
    matmul semaphore dispatch costs ~a full N=512 stream time).
  - scores S^T[k,q]: two k-tiles fused per [128,1024] PSUM tile per head;
    one ACT exp evacuates both; causal mask via multiplicative 0/1 bf16
    masks on diagonal tiles (fully-masked tiles skipped).
  - AV: lhsT = [V | ones x 64] (M=128, same N-bound matmul cost as M=65):
    PSUM rows 0..63 = O^T (unnormalized), rows 64..127 = 64 replicated
    copies of the softmax denominator -> normalization needs no partition
    broadcast at all: a 5-op DVE Newton-reciprocal chain (bitwise seed +
    one Chebyshev-scaled NR step, ~0.17% max err) multiplies O^T in place,
    keeping the normalize entirely off the ACT queue that paces the exp
    chain. Normalize chains are deferred into the next (hp, q-block) group
    so the PE never stalls on them and the DVE PSUM read happens well
    after the AV matmul drain (direct DVE reads of freshly PE-written PSUM
    were observed to return garbage on HW).
  - All PSUM evacuation of projection groups goes through ACT (race-safe).
  - c_proj contracts the stacked O^T [512, S] against W_proj rows; its
    groups are pumped into the pair-3 attention stretch.
"""

import os
import sys

import numpy as np

for _p in ("/opt/trn_rl_repo", "/root/.axon_site/_ro/trn_rl_repo"):
    if os.path.isdir(_p) and _p not in sys.path:
        sys.path.append(_p)

import ml_dtypes  # noqa: E402

import concourse.bass as bass  # noqa: E402
import concourse.tile as tile  # noqa: E402
from concourse import bacc, bass_utils, mybir  # noqa: E402

BF16 = ml_dtypes.bfloat16

B, S, E, H = 4, 2048, 1024, 16
D = E // H            # 64
NCORE = 8
HPC = H // 2          # heads per core = 8
NPAIR = HPC // 2      # head pairs per core = 4
KT = E // 128         # contraction tiles over E = 8
TB512 = S // 512      # 4
TB128 = S // 128      # 16
FPC = HPC * D         # features per core in attention output = 512

_cache: dict = {}


def _ts(i, n):
    return slice(i * n, (i + 1) * n)


def _build_program(with_bias):
    bf = mybir.dt.bfloat16
    f32 = mybir.dt.float32
    nc = bacc.Bacc("TRN2", target_bir_lowering=False, debug=False)

    xt = nc.dram_tensor("xt", [128, KT, S], bf, kind="ExternalInput")
    wqk = nc.dram_tensor("wqk", [128, KT, 2 * FPC], bf, kind="ExternalInput")
    wv = nc.dram_tensor("wv", [128, KT, FPC], bf, kind="ExternalInput")
    wp = nc.dram_tensor("wp", [128, FPC // 128, E], bf, kind="ExternalInput")
    if with_bias:
        bqk = nc.dram_tensor("bqk", [1, 2 * FPC], bf, kind="ExternalInput")
        bv = nc.dram_tensor("bv", [1, FPC], bf, kind="ExternalInput")
    msk = nc.dram_tensor("msk", [128, 4, 512], bf, kind="ExternalInput")
    out = nc.dram_tensor("out", [S, E], f32, kind="ExternalOutput")

    Exp = mybir.ActivationFunctionType.Exp

    with tile.TileContext(nc) as tc:
        with (
            tc.tile_pool(name="big", bufs=1) as big,
            tc.tile_pool(name="pp", bufs=4) as pp,
            tc.tile_pool(name="sm", bufs=2) as sm,
            tc.tile_pool(name="ob", bufs=2) as ob,
        ):
            xts = [
                big.tile([128, S], bf, tag=f"xt{k}", name=f"xt{k}")
                for k in range(KT)
            ]
            wqks = [
                big.tile([128, 2 * FPC], bf, tag=f"wqk{k}", name=f"wqk{k}")
                for k in range(KT)
            ]
            wv_sb = big.tile([128, KT, FPC], bf, tag="wv")
            wp_sb = big.tile([128, FPC // 128, E], bf, tag="wp")
            if with_bias:
                bqk_sb = big.tile([1, 2 * FPC], bf, tag="bqk")
                bv_sb = big.tile([1, FPC], bf, tag="bv")
            msk_sb = big.tile([128, 4, 512], bf, tag="msk")
            if with_bias:
                ones_sb = big.tile([1, S], bf, tag="ones")
            # Q^T per HEAD, zero-padded to 128 partitions (head A data on
            # rows 0..63 with rows 64..127 = 0, head B data on rows 64..127
            # with rows 0..63 = 0). Score matmuls then contract over the full
            # 128 partitions of kTs (both heads) -- the zero rows kill the
            # other head's contribution, the matmul cost is N-bound so the
            # padding is free, and every matmul in the program runs in the
            # same 128-row PE mode (no mode-switch drains between scores and
            # AV/projection work).
            qzs = [big.tile([128, S], bf, tag=f"qz{h}", name=f"qz{h}") for h in range(HPC)]
            # K^T and O^T per pair: head A on partitions 0..63, B on 64..127
            kTs = [big.tile([128, S], bf, tag=f"kT{p}", name=f"kT{p}") for p in range(NPAIR)]
            oTs = [big.tile([128, S], bf, tag=f"oT{p}", name=f"oT{p}") for p in range(NPAIR)]
            # [V | ones*64] stationary tiles: cols 0..63 = V features, cols
            # 64..127 = 1.0 (denominator rows, pre-broadcast in PSUM)
            vone_k = [
                big.tile([128, 2, HPC, 2 * D], bf, tag=f"vone{tp}", name=f"vone{tp}")
                for tp in range(TB128 // 2)
            ]

            if with_bias:
                nc.sync.dma_start(out=bqk_sb, in_=bqk.ap())
                nc.sync.dma_start(out=bv_sb, in_=bv.ap())
            # first token-halves first so phase-1a's tbp=0 groups start early
            for k in range(KT):
                nc.sync.dma_start(out=wqks[k], in_=wqk.ap()[:, k, :])
                nc.sync.dma_start(out=xts[k][:, 0 : S // 2], in_=xt.ap()[:, k, 0 : S // 2])
            nc.sync.dma_start(out=wv_sb, in_=wv.ap())
            nc.sync.dma_start(out=msk_sb, in_=msk.ap())
            for k in range(KT):
                nc.sync.dma_start(
                    out=xts[k][:, S // 2 : S], in_=xt.ap()[:, k, S // 2 : S]
                )
            nc.sync.dma_start(out=wp_sb, in_=wp.ap())

            if with_bias:
                nc.vector.memset(ones_sb, 1.0)
            for tp in range(TB128 // 2):
                nc.vector.memset(vone_k[tp][:, :, :, D : 2 * D], 1.0)
            for h in range(HPC):
                pad = slice(D, 2 * D) if h % 2 == 0 else slice(0, D)
                nc.vector.memset(qzs[h][pad, :], 0.0)

            # Bank budget: sp 2x[128,1024]=4 + av 2x[128,512]=2 +
            # qk 2x[128,512]=2 -> 8 banks exactly.
            with (
                tc.tile_pool(name="sp", bufs=2, space="PSUM") as ps_sp,
                tc.tile_pool(name="av", bufs=2, space="PSUM") as ps_av,
                tc.tile_pool(name="qk", bufs=2, space="PSUM") as ps_qk,
            ):

                # All projection / cproj groups use [128,512] half-group
                # tiles from the 2-buffer ps_qk pool (same 2 PSUM banks as
                # one [128,1024] tile, but ping-ponged): the next group's
                # matmuls overlap the previous group's ACT evacuation, which
                # removes the measured ~1.3us PE stall at every feed-group
                # boundary and pipelines the cproj tail.
                def emit_qk_group(fb, tb):
                    """Half a Q^T/K^T projection group: 8-9 matmuls + evac."""
                    ps = ps_qk.tile([128, 512], f32, tag="qkvi", name=f"qkv_{fb}_{tb}")
                    for kt in range(KT):
                        nc.tensor.matmul(
                            ps,
                            lhsT=wqks[kt][:, _ts(fb, 128)],
                            rhs=xts[kt][:, _ts(tb, 512)],
                            start=(kt == 0),
                            stop=(kt == KT - 1 and not with_bias),
                        )
                        yield
                    if with_bias:
                        nc.tensor.matmul(
                            ps,
                            lhsT=bqk_sb[0:1, _ts(fb, 128)],
                            rhs=ones_sb[0:1, _ts(tb, 512)],
                            start=False,
                            stop=True,
                        )
                        yield
                    if fb < NPAIR:
                        nc.scalar.copy(qzs[2 * fb][0:D, _ts(tb, 512)], ps[0:D, :])
                        yield
                        nc.scalar.copy(
                            qzs[2 * fb + 1][D : 2 * D, _ts(tb, 512)],
                            ps[D : 2 * D, :],
                        )
                    else:
                        nc.scalar.mul(kTs[fb - NPAIR][:, _ts(tb, 512)], ps, 0.125)
                    yield

                # ---- V = x @ Wv (+bias), [token, feature] half-groups ----
                def emit_v_group(tbp, half):
                    tb = 2 * tbp + half
                    ps = ps_qk.tile([128, 512], f32, tag="qkvi", name=f"v_{tb}")
                    for kt in range(KT):
                        nc.tensor.matmul(
                            ps,
                            lhsT=xts[kt][:, _ts(tb, 128)],
                            rhs=wv_sb[:, kt, :],
                            start=(kt == 0),
                            stop=(kt == KT - 1 and not with_bias),
                        )
                        yield
                    if with_bias:
                        nc.tensor.matmul(
                            ps,
                            lhsT=ones_sb[0:1, 0:128],
                            rhs=bv_sb[0:1, :],
                            start=False,
                            stop=True,
                        )
                        yield
                    nc.scalar.copy(
                        vone_k[tbp][:, half, :, 0:D],
                        ps[:, :].rearrange("p (h d) -> p h d", d=D),
                    )
                    yield

                # ---- Phase 1: pair-0 Q^T/K^T upfront (tb-major so the
                # first groups only need the first token-half DMAs), then V
                # upfront (dense PE work) ----
                for tb in range(TB512):
                    for fb in (0, NPAIR):
                        for _ in emit_qk_group(fb, tb):
                            pass
                for tbp in range(TB128 // 2):
                    for half in range(2):
                        for _ in emit_v_group(tbp, half):
                            pass

                # ---- feed: QKV pairs 1..3, interleaved into attention to
                # keep the PE dense while ACT paces the exp chain ----
                def main_feed():
                    for pr in range(1, NPAIR):
                        for fb in (pr, pr + NPAIR):
                            for tb in range(TB512):
                                yield from (
                                    pr for _ in emit_qk_group(fb, tb)
                                )

                feed = main_feed()
                # cur_pr semantics: pairs < cur_pr are fully emitted
                feed_state = {"cur_pr": 1, "alive": True}

                def pump(n):
                    for _ in range(n):
                        got = next(feed, None)
                        if got is None:
                            feed_state["alive"] = False
                            feed_state["cur_pr"] = NPAIR + 1
                            return
                        feed_state["cur_pr"] = got

                def drain_feed_through(pr):
                    """Emit QKV work until every pair <= pr is complete.
                    Required before attention reads qTs[pr]/kTs[pr]: Tile only
                    sees RAW deps for writes emitted before the read."""
                    while feed_state["alive"] and feed_state["cur_pr"] <= pr:
                        pump(1)

                # ---- c_proj feed: pumped into pair-3 attention as the QKV
                # feed runs dry there. A tb group unlocks once every head has
                # normalized its q-block (pair 3 / head B is the last writer).
                def cproj_group(tb, eb):
                    ps = ps_qk.tile([128, 512], f32, tag="qkvi", name=f"pj{tb}_{eb}")
                    for fg in range(FPC // 128):
                        nc.tensor.matmul(
                            ps,
                            lhsT=oTs[fg][:, _ts(tb, 128)],
                            rhs=wp_sb[:, fg, _ts(eb, 512)],
                            start=(fg == 0),
                            stop=(fg == FPC // 128 - 1),
                        )
                        yield
                    o_sb = ob.tile([128, 512], f32, tag="osb", name=f"ob{tb}_{eb}")
                    nc.scalar.copy(o_sb, ps)
                    yield
                    nc.sync.dma_start(
                        out=out.ap()[_ts(tb, 128), _ts(eb, 512)], in_=o_sb
                    )
                    yield

                # tb groups are emitted in pair-3's (descending) qb order so
                # only the tiny qb=0 group's cproj work trails at the tail
                def cproj_feed():
                    for qb_grp in reversed(range(TB512)):
                        while qb_grp not in cp_state["unlocked"]:
                            yield False  # not allowed yet; no emission
                        for tb in range(4 * qb_grp, 4 * qb_grp + 4):
                            for eb in range(2):
                                yield from (True for _ in cproj_group(tb, eb))

                cp_state = {"unlocked": set()}
                cfeed = cproj_feed()

                def pump_cproj(n):
                    for _ in range(n):
                        got = next(cfeed, None)
                        if got is None or got is False:
                            return

                # ---- Phase 2: causal attention per head pair ----
                deferred = []  # pending normalize chains (one per head/group)

                def normalize(av, hp, qoff, qb):
                    # av rows 64..127 hold 64 replicated copies of the
                    # softmax denominator, so no partition broadcast is
                    # needed. 1/denom via a 5-op DVE Newton chain (bitwise
                    # exponent-flip seed + Chebyshev-scaled NR step, ~0.17%
                    # max err) -- keeps the normalize entirely OFF the ACT
                    # queue, which paces the exp chain the PE is coupled to.
                    # All reads of av are deferred a group behind the AV
                    # matmuls (direct DVE reads of freshly PE-written PSUM
                    # return garbage on HW). x = av[D:], num = av[:D].
                    # Ops ordered so every av read is done by op 4: the
                    # final op reads only SBUF, releasing the av bank for
                    # the next group's AV matmuls one DVE op earlier.
                    # oT = num*y0*(c1 - x*y0) = (t - c1)*(-num*y0), t = x*y0.
                    i32 = mybir.dt.int32
                    x = av[D : 2 * D, :]
                    nx = sm.tile([D, 512], f32, tag="nx")
                    y0 = sm.tile([D, 512], f32, tag="y0")
                    tt = sm.tile([D, 512], f32, tag="tt")
                    mm = sm.tile([D, 512], f32, tag="mm")
                    nc.vector.tensor_scalar(
                        nx.bitcast(i32), x.bitcast(i32), -1, None,
                        mybir.AluOpType.bitwise_xor,
                    )
                    nc.vector.tensor_scalar(
                        y0, nx, -0.23549792, None, mybir.AluOpType.mult
                    )
                    nc.vector.tensor_tensor(tt, x, y0, mybir.AluOpType.mult)
                    nc.vector.scalar_tensor_tensor(
                        mm, av[0:D, :], -1.0, y0,
                        mybir.AluOpType.mult, mybir.AluOpType.mult,
                    )
                    nc.vector.scalar_tensor_tensor(
                        oTs[hp][qoff : qoff + D, _ts(qb, 512)],
                        tt, 2.0017324, mm,
                        mybir.AluOpType.subtract, mybir.AluOpType.mult,
                    )

                for hp in range(NPAIR):
                    drain_feed_through(hp)
                    # pair 3 runs its q-blocks big-to-small so the cproj
                    # groups that depend on each qb unlock as early as
                    # possible and only qb=0's trail at the end
                    qb_order = range(TB512)
                    if hp == NPAIR - 1:
                        qb_order = reversed(range(TB512))
                    for qb in qb_order:
                        npairs = 2 * qb + 2  # fused ki-pairs (4qb+4 k-tiles)
                        avs = [
                            ps_av.tile([128, 512], f32, tag="av", name=f"av{hd}")
                            for hd in range(2)
                        ]
                        # pop deferred normalizes BEFORE any AV matmul of this
                        # group: the av pool rotates every group, so the WAR
                        # dep (new AV write vs old normalize read) must see
                        # the read emitted first.
                        while deferred:
                            fn, dhp, dhd, dqb = deferred.pop(0)
                            fn()
                            if dhp == NPAIR - 1 and dhd == 1:
                                cp_state["unlocked"].add(dqb)
                        pend = None

                        def do_av(p, p_sbs, avs=avs, hp=hp, npairs=npairs):
                            for hd in range(2):
                                h = 2 * hp + hd
                                for half in range(2):
                                    ki = 2 * p + half
                                    nc.tensor.matmul(
                                        avs[hd],
                                        lhsT=vone_k[ki // 2][:, ki % 2, h, :],
                                        rhs=p_sbs[hd][:, _ts(half, 512)],
                                        start=(ki == 0),
                                        stop=(ki == 2 * npairs - 1),
                                    )

                        for p in range(npairs):
                            sps = [
                                ps_sp.tile([128, 1024], f32, tag="sp", name=f"sp{hd}")
                                for hd in range(2)
                            ]
                            # K=128 full-partition scores: kTs carries both
                            # heads; the zero-padded qzs tile selects one.
                            # Same N-bound cost as K=64 but no PE mode/row-
                            # group switches anywhere in the program.
                            for hd in range(2):
                                for half in range(2):
                                    ki = 2 * p + half
                                    nc.tensor.matmul(
                                        sps[hd][:, _ts(half, 512)],
                                        lhsT=kTs[hp][:, _ts(ki, 128)],
                                        rhs=qzs[2 * hp + hd][:, _ts(qb, 512)],
                                        start=True,
                                        stop=True,
                                    )
                            if pend is not None:
                                do_av(*pend)
                                pump(4)
                            else:
                                pump(6)
                            p_sbs = [
                                pp.tile([128, 1024], bf, tag="p", name=f"p{hd}")
                                for hd in range(2)
                            ]
                            for hd in range(2):
                                nc.scalar.activation(p_sbs[hd], sps[hd], Exp)
                            if p >= 2 * qb:  # both halves are diagonal tiles
                                j = 2 * (p - 2 * qb)
                                for hd in range(2):
                                    nc.vector.tensor_mul(
                                        p_sbs[hd],
                                        p_sbs[hd],
                                        msk_sb[:, j : j + 2, :].rearrange(
                                            "k j q -> k (j q)"
                                        ),
                                    )
                            pend = (p, p_sbs)
                            if hp == NPAIR - 1:
                                pump_cproj(8)
                        do_av(*pend)
                        for hd in range(2):
                            deferred.append(
                                (
                                    lambda av=avs[hd], hp=hp, qoff=hd * D, qb=qb: normalize(
                                        av, hp, qoff, qb
                                    ),
                                    hp,
                                    hd,
                                    qb,
                                )
                            )
                while feed_state["alive"]:  # drain any remaining feed
                    pump(1)
                while deferred:
                    fn, dhp, dhd, dqb = deferred.pop(0)
                    fn()
                cp_state["unlocked"] = set(range(TB512))
                while next(cfeed, None) is not None:
                    pass

    nc.compile()
    return nc


def _part_major(a, p=128):
    """[n*128, m] -> [128, n, m] with partition index innermost in rows."""
    n = a.shape[0] // p
    return np.ascontiguousarray(a.reshape(n, p, a.shape[1]).transpose(1, 0, 2))


def make_in_maps(x, W_attn, b_attn, W_proj, with_bias=False):
    """Build the 8 per-core input maps (core = 2*b + g)."""
    x = np.asarray(x, dtype=np.float32)
    W_attn = np.asarray(W_attn, dtype=np.float32)
    b_attn = np.asarray(b_attn, dtype=np.float32)
    W_proj = np.asarray(W_proj, dtype=np.float32)

    # causal 0/1 masks for the 4 diagonal alignments (k-tile 128 vs q-block 512)
    kk = np.arange(128)[:, None]
    qq = np.arange(512)[None, :]
    msk = np.stack(
        [(qq >= j * 128 + kk) for j in range(4)], axis=1
    ).astype(BF16)  # [128, 4, 512]

    in_maps = []
    for b in range(B):
        xt = _part_major(np.ascontiguousarray(x[b].T)).astype(BF16)  # [128,8,S]
        for g in range(2):
            qs = W_attn[:, g * FPC : (g + 1) * FPC]
            ks = W_attn[:, E + g * FPC : E + (g + 1) * FPC]
            vs = W_attn[:, 2 * E + g * FPC : 2 * E + (g + 1) * FPC]
            wqk = _part_major(np.concatenate([qs, ks], axis=1)).astype(BF16)
            wv = _part_major(vs).astype(BF16)
            wp = _part_major(W_proj[g * FPC : (g + 1) * FPC, :]).astype(BF16)
            bq = b_attn[g * FPC : (g + 1) * FPC]
            bk = b_attn[E + g * FPC : E + (g + 1) * FPC]
            bqk = np.concatenate([bq, bk])[None, :].astype(BF16)
            bv = b_attn[2 * E + g * FPC : 2 * E + (g + 1) * FPC][None, :].astype(
                BF16
            )
            m = {
                "xt": xt,
                "wqk": np.ascontiguousarray(wqk),
                "wv": np.ascontiguousarray(wv),
                "wp": np.ascontiguousarray(wp),
                "msk": np.ascontiguousarray(msk),
            }
            if with_bias:
                m["bqk"] = np.ascontiguousarray(bqk)
                m["bv"] = np.ascontiguousarray(bv)
            in_maps.append(m)
    return in_maps


def get_program(with_bias=False):
    key = f"nc{int(with_bias)}"
    if key not in _cache:
        _cache[key] = _build_program(with_bias)
    return _cache[key]


def gather(results, b_proj):
    b_proj = np.asarray(b_proj, dtype=np.float32)
    out = np.empty((B, S, E), dtype=np.float32)
    for b in range(B):
        out[b] = results[2 * b]["out"] + results[2 * b + 1]["out"] + b_proj
    return out


def kernel(x, W_attn, b_attn, W_proj, b_proj):
    with_bias = bool(np.any(np.asarray(b_attn)))
    nc = get_program(with_bias)
    in_maps = make_in_maps(x, W_attn, b_attn, W_proj, with_bias=with_bias)
    res = bass_utils.run_bass_kernel_spmd(nc, in_maps, core_ids=list(range(NCORE)))
    return gather(res.results, b_proj)


# revision 32
# speedup vs baseline: 1.2613x; 1.0258x over previous
"""Trainium2 Bass kernel for causal multi-head attention (B=4, S=2048, E=1024, H=16).

Sharding: 8 cores = (batch b in 0..3) x (head-group g in 0..1); each core
computes one batch and 8 heads end-to-end:
  - column-parallel QKV projection (only its heads' columns)
  - causal attention for its 8 heads
  - row-parallel output projection -> partial [S, E]
The two partials per batch are summed on the host (plus b_proj). No on-device
collectives are needed.

Device dataflow (per core), all matmuls in bf16 with fp32 PSUM accumulation:
  - Q^T, K^T computed directly in [feature, token] layout (out = W^T @ x^T),
    so attention needs no transposes. K^T is pre-scaled by 1/sqrt(d)=0.125.
  - Heads are processed in PAIRS: head A lives on SBUF partitions 0..63 of
    kTs/oTs, head B on partitions 64..127. Q^T is stored per-head in
    zero-padded [128,S] tiles so every score matmul contracts the full 128
    partitions (the pad rows multiply to zero): same N-bound cost as K=64,
    but the whole program runs in one PE tiling mode (no mode-switch
    drains; PE row-tile concurrency is unreachable anyway because per-
    matmul semaphore dispatch costs ~a full N=512 stream time).
  - scores S^T[k,q]: two k-tiles fused per [128,1024] PSUM tile per head;
    one ACT exp evacuates both; causal mask via multiplicative 0/1 bf16
    masks on diagonal tiles (fully-masked tiles skipped).
  - AV: lhsT = [V | ones x 64] (M=128, same N-bound matmul cost as M=65):
    PSUM rows 0..63 = O^T (unnormalized), rows 64..127 = 64 replicated
    copies of the softmax denominator -> normalization needs no partition
    broadcast at all: a 5-op DVE Newton-reciprocal chain (bitwise seed +
    one Chebyshev-scaled NR step, ~0.17% max err) multiplies O^T in place,
    keeping the normalize entirely off the ACT queue that paces the exp
    chain. Normalize chains are deferred into the next (hp, q-block) group
    so the PE never stalls on them and the DVE PSUM read happens well
    after the AV matmul drain (direct DVE reads of freshly PE-written PSUM
    were observed to return garbage on HW).
  - All PSUM evacuation of projection groups goes through ACT (race-safe).
  - c_proj contracts the stacked O^T [512, S] against W_proj rows; its
    groups are pumped into the pair-3 attention stretch.
"""

import os
import sys

import numpy as np

for _p in ("/opt/trn_rl_repo", "/root/.axon_site/_ro/trn_rl_repo"):
    if os.path.isdir(_p) and _p not in sys.path:
        sys.path.append(_p)

import ml_dtypes  # noqa: E402

import concourse.bass as bass  # noqa: E402
import concourse.tile as tile  # noqa: E402
from concourse import bacc, bass_utils, mybir  # noqa: E402

BF16 = ml_dtypes.bfloat16

B, S, E, H = 4, 2048, 1024, 16
D = E // H            # 64
NCORE = 8
HPC = H // 2          # heads per core = 8
NPAIR = HPC // 2      # head pairs per core = 4
KT = E // 128         # contraction tiles over E = 8
TB512 = S // 512      # 4
TB128 = S // 128      # 16
FPC = HPC * D         # features per core in attention output = 512

_cache: dict = {}


def _ts(i, n):
    return slice(i * n, (i + 1) * n)


def _build_program(with_bias):
    bf = mybir.dt.bfloat16
    f32 = mybir.dt.float32
    nc = bacc.Bacc("TRN2", target_bir_lowering=False, debug=False)

    xt = nc.dram_tensor("xt", [128, KT, S], bf, kind="ExternalInput")
    wqk = nc.dram_tensor("wqk", [128, KT, 2 * FPC], bf, kind="ExternalInput")
    wv = nc.dram_tensor("wv", [128, KT, FPC], bf, kind="ExternalInput")
    wp = nc.dram_tensor("wp", [128, FPC // 128, E], bf, kind="ExternalInput")
    if with_bias:
        bqk = nc.dram_tensor("bqk", [1, 2 * FPC], bf, kind="ExternalInput")
        bv = nc.dram_tensor("bv", [1, FPC], bf, kind="ExternalInput")
    msk = nc.dram_tensor("msk", [128, 4, 512], bf, kind="ExternalInput")
    out = nc.dram_tensor("out", [S, E], f32, kind="ExternalOutput")

    Exp = mybir.ActivationFunctionType.Exp

    with tile.TileContext(nc) as tc:
        with (
            tc.tile_pool(name="big", bufs=1) as big,
            tc.tile_pool(name="pp", bufs=4) as pp,
            tc.tile_pool(name="sm", bufs=2) as sm,
            tc.tile_pool(name="ob", bufs=4) as ob,
        ):
            xts = [
                big.tile([128, S], bf, tag=f"xt{k}", name=f"xt{k}")
                for k in range(KT)
            ]
            wqks = [
                big.tile([128, 2 * FPC], bf, tag=f"wqk{k}", name=f"wqk{k}")
                for k in range(KT)
            ]
            wv_sb = big.tile([128, KT, FPC], bf, tag="wv")
            wp_sb = big.tile([128, FPC // 128, E], bf, tag="wp")
            if with_bias:
                bqk_sb = big.tile([1, 2 * FPC], bf, tag="bqk")
                bv_sb = big.tile([1, FPC], bf, tag="bv")
            msk_sb = big.tile([128, 4, 512], bf, tag="msk")
            if with_bias:
                ones_sb = big.tile([1, S], bf, tag="ones")
            # Q^T per HEAD, zero-padded to 128 partitions (head A data on
            # rows 0..63 with rows 64..127 = 0, head B data on rows 64..127
            # with rows 0..63 = 0). Score matmuls then contract over the full
            # 128 partitions of kTs (both heads) -- the zero rows kill the
            # other head's contribution, the matmul cost is N-bound so the
            # padding is free, and every matmul in the program runs in the
            # same 128-row PE mode (no mode-switch drains between scores and
            # AV/projection work).
            qzs = [big.tile([128, S], bf, tag=f"qz{h}", name=f"qz{h}") for h in range(HPC)]
            # K^T and O^T per pair: head A on partitions 0..63, B on 64..127
            kTs = [big.tile([128, S], bf, tag=f"kT{p}", name=f"kT{p}") for p in range(NPAIR)]
            oTs = [big.tile([128, S], bf, tag=f"oT{p}", name=f"oT{p}") for p in range(NPAIR)]
            # [V | ones*64] stationary tiles: cols 0..63 = V features, cols
            # 64..127 = 1.0 (denominator rows, pre-broadcast in PSUM)
            vone_k = [
                big.tile([128, 2, HPC, 2 * D], bf, tag=f"vone{tp}", name=f"vone{tp}")
                for tp in range(TB128 // 2)
            ]

            if with_bias:
                nc.sync.dma_start(out=bqk_sb, in_=bqk.ap())
                nc.sync.dma_start(out=bv_sb, in_=bv.ap())
            # first token-halves first so the tb0/tb1 QK groups start early;
            # wv mid-stream so the V groups that follow them aren't stalled
            for k in range(KT):
                nc.sync.dma_start(out=wqks[k], in_=wqk.ap()[:, k, :])
                nc.sync.dma_start(out=xts[k][:, 0 : S // 2], in_=xt.ap()[:, k, 0 : S // 2])
                if k == 5:
                    nc.sync.dma_start(out=wv_sb, in_=wv.ap())
            nc.sync.dma_start(out=msk_sb, in_=msk.ap())
            for k in range(KT):
                nc.sync.dma_start(
                    out=xts[k][:, S // 2 : S], in_=xt.ap()[:, k, S // 2 : S]
                )
            nc.sync.dma_start(out=wp_sb, in_=wp.ap())

            if with_bias:
                nc.vector.memset(ones_sb, 1.0)
            for tp in range(TB128 // 2):
                nc.vector.memset(vone_k[tp][:, :, :, D : 2 * D], 1.0)
            for h in range(HPC):
                pad = slice(D, 2 * D) if h % 2 == 0 else slice(0, D)
                nc.vector.memset(qzs[h][pad, :], 0.0)

            # Bank budget: sp 2x[128,1024]=4 + av 2x[128,512]=2 +
            # qk 2x[128,512]=2 -> 8 banks exactly.
            with (
                tc.tile_pool(name="sp", bufs=2, space="PSUM") as ps_sp,
                tc.tile_pool(name="av", bufs=2, space="PSUM") as ps_av,
                tc.tile_pool(name="qk", bufs=2, space="PSUM") as ps_qk,
            ):

                # All projection / cproj groups use [128,512] half-group
                # tiles from the 2-buffer ps_qk pool (same 2 PSUM banks as
                # one [128,1024] tile, but ping-ponged): the next group's
                # matmuls overlap the previous group's ACT evacuation, which
                # removes the measured ~1.3us PE stall at every feed-group
                # boundary and pipelines the cproj tail.
                def emit_qk_group(fb, tb):
                    """Half a Q^T/K^T projection group: 8-9 matmuls + evac."""
                    ps = ps_qk.tile([128, 512], f32, tag="qkvi", name=f"qkv_{fb}_{tb}")
                    for kt in range(KT):
                        nc.tensor.matmul(
                            ps,
                            lhsT=wqks[kt][:, _ts(fb, 128)],
                            rhs=xts[kt][:, _ts(tb, 512)],
                            start=(kt == 0),
                            stop=(kt == KT - 1 and not with_bias),
                        )
                        yield
                    if with_bias:
                        nc.tensor.matmul(
                            ps,
                            lhsT=bqk_sb[0:1, _ts(fb, 128)],
                            rhs=ones_sb[0:1, _ts(tb, 512)],
                            start=False,
                            stop=True,
                        )
                        yield
                    if fb < NPAIR:
                        nc.scalar.copy(qzs[2 * fb][0:D, _ts(tb, 512)], ps[0:D, :])
                        yield
                        nc.scalar.copy(
                            qzs[2 * fb + 1][D : 2 * D, _ts(tb, 512)],
                            ps[D : 2 * D, :],
                        )
                    else:
                        nc.scalar.mul(kTs[fb - NPAIR][:, _ts(tb, 512)], ps, 0.125)
                    yield

                # ---- V = x @ Wv (+bias), [token, feature] half-groups ----
                def emit_v_group(tbp, half):
                    tb = 2 * tbp + half
                    ps = ps_qk.tile([128, 512], f32, tag="qkvi", name=f"v_{tb}")
                    for kt in range(KT):
                        nc.tensor.matmul(
                            ps,
                            lhsT=xts[kt][:, _ts(tb, 128)],
                            rhs=wv_sb[:, kt, :],
                            start=(kt == 0),
                            stop=(kt == KT - 1 and not with_bias),
                        )
                        yield
                    if with_bias:
                        nc.tensor.matmul(
                            ps,
                            lhsT=ones_sb[0:1, 0:128],
                            rhs=bv_sb[0:1, :],
                            start=False,
                            stop=True,
                        )
                        yield
                    nc.scalar.copy(
                        vone_k[tbp][:, half, :, 0:D],
                        ps[:, :].rearrange("p (h d) -> p h d", d=D),
                    )
                    yield

                # ---- Phase 1, ordered by DMA arrival: tb0/tb1 QK groups
                # and V groups 0..3 only need the first token-half of x, so
                # they run while the second half streams in ----
                for tb in (0, 1):
                    for fb in (0, NPAIR):
                        for _ in emit_qk_group(fb, tb):
                            pass
                for tbp in (0, 1, 2, 3):
                    for half in range(2):
                        for _ in emit_v_group(tbp, half):
                            pass
                for tb in (2, 3):
                    for fb in (0, NPAIR):
                        for _ in emit_qk_group(fb, tb):
                            pass
                for tbp in (4, 5, 6, 7):
                    for half in range(2):
                        for _ in emit_v_group(tbp, half):
                            pass

                # ---- feed: QKV pairs 1..3, interleaved into attention to
                # keep the PE dense while ACT paces the exp chain ----
                def main_feed():
                    for pr in range(1, NPAIR):
                        for fb in (pr, pr + NPAIR):
                            for tb in range(TB512):
                                yield from (
                                    pr for _ in emit_qk_group(fb, tb)
                                )

                feed = main_feed()
                # cur_pr semantics: pairs < cur_pr are fully emitted
                feed_state = {"cur_pr": 1, "alive": True}

                def pump(n):
                    for _ in range(n):
                        got = next(feed, None)
                        if got is None:
                            feed_state["alive"] = False
                            feed_state["cur_pr"] = NPAIR + 1
                            return
                        feed_state["cur_pr"] = got

                def drain_feed_through(pr):
                    """Emit QKV work until every pair <= pr is complete.
                    Required before attention reads qTs[pr]/kTs[pr]: Tile only
                    sees RAW deps for writes emitted before the read."""
                    while feed_state["alive"] and feed_state["cur_pr"] <= pr:
                        pump(1)

                # ---- c_proj feed: pumped into pair-3 attention as the QKV
                # feed runs dry there. A tb group unlocks once every head has
                # normalized its q-block (pair 3 / head B is the last writer).
                def cproj_group(tb, eb):
                    ps = ps_qk.tile([128, 512], f32, tag="qkvi", name=f"pj{tb}_{eb}")
                    for fg in range(FPC // 128):
                        nc.tensor.matmul(
                            ps,
                            lhsT=oTs[fg][:, _ts(tb, 128)],
                            rhs=wp_sb[:, fg, _ts(eb, 512)],
                            start=(fg == 0),
                            stop=(fg == FPC // 128 - 1),
                        )
                        yield
                    o_sb = ob.tile([128, 512], f32, tag="osb", name=f"ob{tb}_{eb}")
                    nc.scalar.copy(o_sb, ps)
                    yield
                    nc.sync.dma_start(
                        out=out.ap()[_ts(tb, 128), _ts(eb, 512)], in_=o_sb
                    )
                    yield

                # tb groups are emitted in pair-3's (descending) qb order so
                # only the tiny qb=0 group's cproj work trails at the tail
                def cproj_feed():
                    for qb_grp in reversed(range(TB512)):
                        while qb_grp not in cp_state["unlocked"]:
                            yield False  # not allowed yet; no emission
                        for tb in range(4 * qb_grp, 4 * qb_grp + 4):
                            for eb in range(2):
                                yield from (True for _ in cproj_group(tb, eb))

                cp_state = {"unlocked": set()}
                cfeed = cproj_feed()

                def pump_cproj(n):
                    for _ in range(n):
                        got = next(cfeed, None)
                        if got is None or got is False:
                            return

                # ---- Phase 2: causal attention per head pair ----
                deferred = []  # pending normalize chains (one per head/group)

                def normalize(av, hp, qoff, qb):
                    # av rows 64..127 hold 64 replicated copies of the
                    # softmax denominator, so no partition broadcast is
                    # needed. 1/denom via a 5-op DVE Newton chain (bitwise
                    # exponent-flip seed + Chebyshev-scaled NR step, ~0.17%
                    # max err) -- keeps the normalize entirely OFF the ACT
                    # queue, which paces the exp chain the PE is coupled to.
                    # All reads of av are deferred a group behind the AV
                    # matmuls (direct DVE reads of freshly PE-written PSUM
                    # return garbage on HW). x = av[D:], num = av[:D].
                    # Ops ordered so every av read is done by op 4: the
                    # final op reads only SBUF, releasing the av bank for
                    # the next group's AV matmuls one DVE op earlier.
                    # oT = num*y0*(c1 - x*y0) = (t - c1)*(-num*y0), t = x*y0.
                    i32 = mybir.dt.int32
                    x = av[D : 2 * D, :]
                    nx = sm.tile([D, 512], f32, tag="nx")
                    y0 = sm.tile([D, 512], f32, tag="y0")
                    tt = sm.tile([D, 512], f32, tag="tt")
                    mm = sm.tile([D, 512], f32, tag="mm")
                    nc.vector.tensor_scalar(
                        nx.bitcast(i32), x.bitcast(i32), -1, None,
                        mybir.AluOpType.bitwise_xor,
                    )
                    nc.vector.tensor_scalar(
                        y0, nx, -0.23549792, None, mybir.AluOpType.mult
                    )
                    nc.vector.tensor_tensor(tt, x, y0, mybir.AluOpType.mult)
                    nc.vector.scalar_tensor_tensor(
                        mm, av[0:D, :], -1.0, y0,
                        mybir.AluOpType.mult, mybir.AluOpType.mult,
                    )
                    nc.vector.scalar_tensor_tensor(
                        oTs[hp][qoff : qoff + D, _ts(qb, 512)],
                        tt, 2.0017324, mm,
                        mybir.AluOpType.subtract, mybir.AluOpType.mult,
                    )

                for hp in range(NPAIR):
                    drain_feed_through(hp)
                    # pair 3 runs its q-blocks big-to-small so the cproj
                    # groups that depend on each qb unlock as early as
                    # possible and only qb=0's trail at the end
                    qb_order = range(TB512)
                    if hp == NPAIR - 1:
                        qb_order = reversed(range(TB512))
                    for qb in qb_order:
                        npairs = 2 * qb + 2  # fused ki-pairs (4qb+4 k-tiles)
                        avs = [
                            ps_av.tile([128, 512], f32, tag="av", name=f"av{hd}")
                            for hd in range(2)
                        ]
                        # pop deferred normalizes BEFORE any AV matmul of this
                        # group: the av pool rotates every group, so the WAR
                        # dep (new AV write vs old normalize read) must see
                        # the read emitted first.
                        while deferred:
                            fn, dhp, dhd, dqb = deferred.pop(0)
                            fn()
                            if dhp == NPAIR - 1 and dhd == 1:
                                cp_state["unlocked"].add(dqb)
                        pend = None

                        def do_av(p, p_sbs, avs=avs, hp=hp, npairs=npairs):
                            for hd in range(2):
                                h = 2 * hp + hd
                                for half in range(2):
                                    ki = 2 * p + half
                                    nc.tensor.matmul(
                                        avs[hd],
                                        lhsT=vone_k[ki // 2][:, ki % 2, h, :],
                                        rhs=p_sbs[hd][:, _ts(half, 512)],
                                        start=(ki == 0),
                                        stop=(ki == 2 * npairs - 1),
                                    )

                        for p in range(npairs):
                            sps = [
                                ps_sp.tile([128, 1024], f32, tag="sp", name=f"sp{hd}")
                                for hd in range(2)
                            ]
                            # K=128 full-partition scores: kTs carries both
                            # heads; the zero-padded qzs tile selects one.
                            # Same N-bound cost as K=64 but no PE mode/row-
                            # group switches anywhere in the program.
                            for hd in range(2):
                                for half in range(2):
                                    ki = 2 * p + half
                                    nc.tensor.matmul(
                                        sps[hd][:, _ts(half, 512)],
                                        lhsT=kTs[hp][:, _ts(ki, 128)],
                                        rhs=qzs[2 * hp + hd][:, _ts(qb, 512)],
                                        start=True,
                                        stop=True,
                                    )
                            if pend is not None:
                                do_av(*pend)
                                pump(4)
                            else:
                                pump(6)
                            p_sbs = [
                                pp.tile([128, 1024], bf, tag="p", name=f"p{hd}")
                                for hd in range(2)
                            ]
                            for hd in range(2):
                                nc.scalar.activation(p_sbs[hd], sps[hd], Exp)
                            if p >= 2 * qb:  # both halves are diagonal tiles
                                j = 2 * (p - 2 * qb)
                                for hd in range(2):
                                    nc.vector.tensor_mul(
                                        p_sbs[hd],
                                        p_sbs[hd],
                                        msk_sb[:, j : j + 2, :].rearrange(
                                            "k j q -> k (j q)"
                                        ),
                                    )
                            pend = (p, p_sbs)
                            if hp == NPAIR - 1:
                                pump_cproj(8)
                        do_av(*pend)
                        for hd in range(2):
                            deferred.append(
                                (
                                    lambda av=avs[hd], hp=hp, qoff=hd * D, qb=qb: normalize(
                                        av, hp, qoff, qb
                                    ),
                                    hp,
                                    hd,
                                    qb,
                                )
                            )
                while feed_state["alive"]:  # drain any remaining feed
                    pump(1)
                while deferred:
                    fn, dhp, dhd, dqb = deferred.pop(0)
                    fn()
                cp_state["unlocked"] = set(range(TB512))
                while next(cfeed, None) is not None:
                    pass

    nc.compile()
    return nc


def _part_major(a, p=128):
    """[n*128, m] -> [128, n, m] with partition index innermost in rows."""
    n = a.shape[0] // p
    return np.ascontiguousarray(a.reshape(n, p, a.shape[1]).transpose(1, 0, 2))


def make_in_maps(x, W_attn, b_attn, W_proj, with_bias=False):
    """Build the 8 per-core input maps (core = 2*b + g)."""
    x = np.asarray(x, dtype=np.float32)
    W_attn = np.asarray(W_attn, dtype=np.float32)
    b_attn = np.asarray(b_attn, dtype=np.float32)
    W_proj = np.asarray(W_proj, dtype=np.float32)

    # causal 0/1 masks for the 4 diagonal alignments (k-tile 128 vs q-block 512)
    kk = np.arange(128)[:, None]
    qq = np.arange(512)[None, :]
    msk = np.stack(
        [(qq >= j * 128 + kk) for j in range(4)], axis=1
    ).astype(BF16)  # [128, 4, 512]

    in_maps = []
    for b in range(B):
        xt = _part_major(np.ascontiguousarray(x[b].T)).astype(BF16)  # [128,8,S]
        for g in range(2):
            qs = W_attn[:, g * FPC : (g + 1) * FPC]
            ks = W_attn[:, E + g * FPC : E + (g + 1) * FPC]
            vs = W_attn[:, 2 * E + g * FPC : 2 * E + (g + 1) * FPC]
            wqk = _part_major(np.concatenate([qs, ks], axis=1)).astype(BF16)
            wv = _part_major(vs).astype(BF16)
            wp = _part_major(W_proj[g * FPC : (g + 1) * FPC, :]).astype(BF16)
            bq = b_attn[g * FPC : (g + 1) * FPC]
            bk = b_attn[E + g * FPC : E + (g + 1) * FPC]
            bqk = np.concatenate([bq, bk])[None, :].astype(BF16)
            bv = b_attn[2 * E + g * FPC : 2 * E + (g + 1) * FPC][None, :].astype(
                BF16
            )
            m = {
                "xt": xt,
                "wqk": np.ascontiguousarray(wqk),
                "wv": np.ascontiguousarray(wv),
                "wp": np.ascontiguousarray(wp),
                "msk": np.ascontiguousarray(msk),
            }
            if with_bias:
                m["bqk"] = np.ascontiguousarray(bqk)
                m["bv"] = np.ascontiguousarray(bv)
            in_maps.append(m)
    return in_maps


def get_program(with_bias=False):
    key = f"nc{int(with_bias)}"
    if key not in _cache:
        _cache[key] = _build_program(with_bias)
    return _cache[key]


def gather(results, b_proj):
    b_proj = np.asarray(b_proj, dtype=np.float32)
    out = np.empty((B, S, E), dtype=np.float32)
    for b in range(B):
        out[b] = results[2 * b]["out"] + results[2 * b + 1]["out"] + b_proj
    return out


def kernel(x, W_attn, b_attn, W_proj, b_proj):
    with_bias = bool(np.any(np.asarray(b_attn)))
    nc = get_program(with_bias)
    in_maps = make_in_maps(x, W_attn, b_attn, W_proj, with_bias=with_bias)
    res = bass_utils.run_bass_kernel_spmd(nc, in_maps, core_ids=list(range(NCORE)))
    return gather(res.results, b_proj)
